# revision 18
# baseline (speedup 1.0000x reference)
"""Trainium2 Bass kernel for per-position multi-head "attention across heads".

Reference math (per position r):
    Q = x @ Wq.T ; K = x @ Wk.T ; V = x @ Wv.T          (H=1024, nh=16, hd=64)
    scores[r, i, j] = (1/8) * sum_d Q[r,i,d] * K[r,j,d]   -> [nh, nh] per position
    attn = softmax(scores, axis=-1)
    out[r, i, :] = sum_j attn[r,i,j] * V[r, j, :]

Strategy (8 NeuronCores, data-parallel over the 8192 = B*L positions):
  - Each core handles R=1024 positions: x_shard [1024, 1024] plus full Wq/Wk/Wv.
  - bf16 compute on the TensorEngine (PSUM accumulation in fp32).
  - x^T and W^T produced by PE transposes (fp32 has no DMA transpose path).
  - Projections produce Q^T/K^T/V^T in a "head-major" SBUF layout
    hm[d (64 partitions), head, r] so that per-position [d, head] operand
    tiles are just strided APs.
  - Scores for 8 positions at a time via ONE K=64 matmul:
      lhsT = K^T strided [64, (8 pos, 16 j)], rhs = Q^T strided [64, (8 pos, 16 i)]
      -> PSUM [ (pos,j), (pos,i) ] with garbage off-diagonal blocks.
  - exp via ScalarE (no max subtraction needed: |scores| <= ~3), mask off-diag
    garbage blocks with a precomputed block mask, then
  - AV via matmul with contraction over (pos, j): lhsT = V_stack [ (pos,j), d ]
    (built by a strided PE transpose), rhs = masked exp. A second 1-row matmul
    with a ones lhsT accumulates the softmax denominators into PSUM row 64.
  - PE-transpose the [65, 128] result back to [ (pos,i), d|Z ] layout, then
    normalize rows by 1/Z on the VectorE and DMA straight to HBM.
"""

import numpy as np

import concourse.bass as bass
import concourse.mybir as mybir
import concourse.tile as tile
from concourse import bacc

F32 = mybir.dt.float32
BF16 = mybir.dt.bfloat16

B, L, H = 4, 2048, 1024
NH, HD = 16, 64
P = 128
N_CORES = 8
R = (B * L) // N_CORES          # positions per core = 1024
KC = H // P                     # contraction chunks = 8
OC = H // P                     # output-feature chunks = 8
GS = 8                          # positions per attention group
GB = 4                          # groups per PSUM-bank batch
SCALE = 1.0 / np.sqrt(HD)


def build_nc(r_core=R):
    RC = r_core
    RT = RC // P                # x row tiles
    NGRP = RC // GS             # attention groups
    NBATCH = NGRP // GB         # group batches

    nc = bacc.Bacc(None, target_bir_lowering=False, debug=False)

    x = nc.dram_tensor("x", [RC, H], F32, kind="ExternalInput")
    Ws = {m: nc.dram_tensor(f"W{m}", [H, H], F32, kind="ExternalInput")
          for m in ("q", "k", "v")}
    ident_bf_d = nc.dram_tensor("ident_bf", [P, P], BF16, kind="ExternalInput")
    ident_f32_d = nc.dram_tensor("ident_f32", [P, P], F32, kind="ExternalInput")
    blkmask_d = nc.dram_tensor("blkmask", [P, P], BF16, kind="ExternalInput")
    ones_col_d = nc.dram_tensor("ones_col", [P, 1], BF16, kind="ExternalInput")
    out = nc.dram_tensor("out", [RC, H], F32, kind="ExternalOutput")

    with tile.TileContext(nc) as tc:
        with tc.tile_pool(name="const", bufs=1) as constp, \
             tc.tile_pool(name="persist", bufs=1) as persist:
            ident_bf = constp.tile([P, P], BF16)
            ident_f32 = constp.tile([P, P], F32)
            blkmask = constp.tile([P, P], BF16)
            ones_col = constp.tile([P, 1], BF16)
            nc.sync.dma_start(ident_bf[:], ident_bf_d[:])
            nc.sync.dma_start(ident_f32[:], ident_f32_d[:])
            nc.sync.dma_start(blkmask[:], blkmask_d[:])
            nc.sync.dma_start(ones_col[:], ones_col_d[:])

            # persistent big tensors
            xT = persist.tile([P, KC, RC], BF16)               # x^T chunks
            # Q^T/K^T/V^T position-major [d, r, head]: matmul operand APs
            # must have a single free dimension, and [d, r0:r0+8, :] is a
            # contiguous 128-wide slice in this layout.
            pm = {m: persist.tile([64, RC, NH], BF16, name=f"pm_{m}")
                  for m in ("q", "k", "v")}

            # ---- phase 0+1: load x (cast bf16) and PE-transpose it ----
            with tc.tile_pool(name="xnat", bufs=1) as xnatp, \
                 tc.tile_pool(name="xtps", bufs=2, space="PSUM") as xtpsp:
                x_sb = xnatp.tile([P, RT, H], BF16)
                nc.gpsimd.dma_start(
                    x_sb[:], x.rearrange("(rt p) h -> p rt h", p=P))
                for kc in range(KC):
                    pt = xtpsp.tile([P, RT, P], BF16)
                    for rt in range(RT):
                        nc.tensor.matmul(
                            pt[:, rt, :], x_sb[:, rt, kc * P:(kc + 1) * P],
                            ident_bf[:], is_transpose=True,
                            start=(rt == 0), stop=(rt == RT - 1))
                    nc.scalar.copy(
                        xT[:, kc, :],
                        pt[:].rearrange("p a b -> p (a b)"))

            # ---- phase 2: W loads, W^T transposes, projections ----
            with tc.tile_pool(name="wnat", bufs=1) as wnatp, \
                 tc.tile_pool(name="wtps", bufs=2, space="PSUM") as wtpsp, \
                 tc.tile_pool(name="wT", bufs=3) as wTp, \
                 tc.tile_pool(name="oddstg", bufs=2) as oddstg, \
                 tc.tile_pool(name="projps", bufs=2, space="PSUM") as projpsp:
                w_sb = {}
                for m in ("q", "k", "v"):
                    w_sb[m] = wnatp.tile([P, OC, H], BF16, tag=f"w_{m}", name=f"w_{m}")
                    nc.gpsimd.dma_start(
                        w_sb[m][:], Ws[m].rearrange("(oc p) k -> p oc k", p=P))
                for m in ("q", "k", "v"):
                    for oc in range(OC):
                        wtp = wtpsp.tile([P, KC, P], BF16)
                        for kc in range(KC):
                            nc.tensor.matmul(
                                wtp[:, kc, :],
                                w_sb[m][:, oc, kc * P:(kc + 1) * P],
                                ident_bf[:], is_transpose=True,
                                start=(kc == 0), stop=(kc == KC - 1))
                        wT = wTp.tile([P, KC, P], BF16)
                        nc.scalar.copy(wT[:], wtp[:])
                        RH = max(1, RC // 512)
                        NRH = RC // RH
                        pp = projpsp.tile([P, RH, NRH], F32)
                        for rh in range(RH):
                            for kc in range(KC):
                                nc.tensor.matmul(
                                    pp[:, rh, :],
                                    wT[:, kc, :],
                                    xT[:, kc, rh * NRH:(rh + 1) * NRH],
                                    start=(kc == 0), stop=(kc == KC - 1))
                        # evict into position-major layout (cast to bf16):
                        # even head (2*oc) comes from PSUM partitions 0-63
                        nc.vector.tensor_copy(
                            pm[m][:, :, 2 * oc],
                            pp[0:64, :, :].rearrange("p a b -> p (a b)"))
                        # odd head (2*oc+1): DVE evict (partition-preserving),
                        # SBUF->SBUF DMA partition shift 64-127 -> 0-63, then
                        # an ACT strided scatter into the pm column.
                        stg = oddstg.tile([P, RC], BF16, tag="oddstg")
                        nc.vector.tensor_copy(
                            stg[64:128, :],
                            pp[64:128, :, :].rearrange("p a b -> p (a b)"))
                        od = oddstg.tile([64, RC], BF16, tag="od")
                        nc.sync.dma_start(od[:], stg[64:128, :])
                        nc.scalar.copy(pm[m][:, :, 2 * oc + 1], od[:])

            # ---- phase 3: attention ----
            with tc.tile_pool(name="sps", bufs=2, space="PSUM") as spsp, \
                 tc.tile_pool(name="vps", bufs=2, space="PSUM") as vpsp, \
                 tc.tile_pool(name="avps", bufs=2, space="PSUM") as avpsp, \
                 tc.tile_pool(name="tps", bufs=2, space="PSUM") as tpsp, \
                 tc.tile_pool(name="att", bufs=3) as attp:
                for b in range(NBATCH):
                    ps = spsp.tile([P, GB, P], F32)
                    pv = vpsp.tile([P, GB, HD], BF16)
                    for g4 in range(GB):
                        r0 = (b * GB + g4) * GS
                        kap = pm["k"][:, r0:r0 + GS, :].rearrange("p s j -> p (s j)")
                        qap = pm["q"][:, r0:r0 + GS, :].rearrange("p s i -> p (s i)")
                        vap = pm["v"][:, r0:r0 + GS, :].rearrange("p s j -> p (s j)")
                        nc.tensor.matmul(
                            ps[:, g4, :], kap, qap,
                            start=(g4 == 0), stop=(g4 == GB - 1))
                        nc.tensor.matmul(
                            pv[:, g4, :], vap, ident_bf[0:64, 0:HD],
                            is_transpose=True,
                            start=(g4 == 0), stop=(g4 == GB - 1))
                    E = attp.tile([P, GB, P], BF16, tag="E")
                    nc.scalar.activation(
                        E[:], ps[:], mybir.ActivationFunctionType.Exp,
                        scale=float(SCALE))
                    Em = attp.tile([P, GB, P], BF16, tag="Em")
                    nc.vector.tensor_tensor(
                        Em[:], E[:],
                        blkmask[:, None, :].to_broadcast((P, GB, P)),
                        mybir.AluOpType.mult)
                    Vs = attp.tile([P, GB, HD], BF16, tag="Vs")
                    nc.vector.tensor_copy(Vs[:], pv[:])
                    pav = avpsp.tile([65, GB, P], F32)
                    for g4 in range(GB):
                        nc.tensor.matmul(
                            pav[0:64, g4, :], Vs[:, g4, :], Em[:, g4, :],
                            start=(g4 == 0), stop=(g4 == GB - 1))
                        # Z row (partition 64) is disjoint from the AV rows;
                        # group bookkeeping tracks it separately.
                        nc.tensor.matmul(
                            pav[64:65, g4, :], ones_col[:], Em[:, g4, :],
                            start=True, stop=True, skip_group_check=True)
                    av = attp.tile([65, GB, P], F32, tag="av")
                    nc.scalar.copy(av[:], pav[:])
                    pt = tpsp.tile([P, GB, 65], F32)
                    for g4 in range(GB):
                        nc.tensor.matmul(
                            pt[:, g4, :], av[:, g4, :], ident_f32[0:65, 0:65],
                            is_transpose=True,
                            start=(g4 == 0), stop=(g4 == GB - 1))
                    o_sb = attp.tile([P, GB, 65], F32, tag="o_sb")
                    nc.scalar.copy(o_sb[:], pt[:])
                    rz = attp.tile([P, GB], F32, tag="rz")
                    nc.vector.reciprocal(rz[:], o_sb[:, :, 64])
                    o_nrm = attp.tile([P, GB, HD], F32, tag="o_nrm")
                    nc.vector.tensor_tensor(
                        o_nrm[:], o_sb[:, :, 0:HD],
                        rz[:, :, None].to_broadcast((P, GB, HD)),
                        mybir.AluOpType.mult)
                    # store: partition (s,i) + free (g4, d) -> rows b*32+g4*8+s
                    for s in range(GS):
                        nc.sync.dma_start(
                            out[b * GB * GS + s:(b + 1) * GB * GS:GS, :]
                            .rearrange("g (i d) -> i g d", i=NH),
                            o_nrm[s * NH:(s + 1) * NH, :, :])

    nc.compile()
    return nc


def _consts():
    import ml_dtypes
    ident = np.eye(P)
    blk = np.kron(np.eye(GS), np.ones((NH, NH)))
    return {
        "ident_bf": ident.astype(ml_dtypes.bfloat16),
        "ident_f32": ident.astype(np.float32),
        "blkmask": blk.astype(ml_dtypes.bfloat16),
        "ones_col": np.ones((P, 1), dtype=ml_dtypes.bfloat16),
    }


_NC_CACHE = {}


def kernel(x, Wq, Wk, Wv):
    from concourse.bass_utils import run_bass_kernel_spmd

    x = np.ascontiguousarray(np.asarray(x, dtype=np.float32))
    xf = x.reshape(B * L, H)
    consts = _consts()
    Wd = {"Wq": np.asarray(Wq, np.float32), "Wk": np.asarray(Wk, np.float32),
          "Wv": np.asarray(Wv, np.float32)}
    in_maps = []
    for c in range(N_CORES):
        m = {"x": np.ascontiguousarray(xf[c * R:(c + 1) * R]),
             "Wq": Wd["Wq"], "Wk": Wd["Wk"], "Wv": Wd["Wv"]}
        m.update(consts)
        in_maps.append(m)

    if "nc" not in _NC_CACHE:
        _NC_CACHE["nc"] = build_nc()
    res = run_bass_kernel_spmd(_NC_CACHE["nc"], in_maps,
                               core_ids=list(range(N_CORES)))
    outs = [r["out"] for r in res.results]
    return np.concatenate(outs, axis=0).reshape(B, L, H).astype(np.float32)
